# revision 14
# baseline (speedup 1.0000x reference)
"""Trainium2 Bass kernel for nn_MHSA_40346922778634.

Math (per batch b, head h; the reference computes-then-drops the register
group, so reg_qk/reg_v are dead inputs):
  X = x[b] as [C=512, N=1024]
  Q = Wq X + bq ; K = Wk X + bk ; V = Wv X + bv       (per head: [64, N])
  P_h = (rel_h + rel_w) reshaped [head, 64, N]
  E[i,j] = Q_h[:,i].K_h[:,j] + P_h[:,i].Q_h[:,j]      ([N, N])
  attn = softmax(E, axis=-1)
  Out_h = V_h @ attn^T ; out[b, h*64:(h+1)*64] = Out_h + X[h*64:(h+1)*64]

Kernel strategy (8 cores, data-parallel over batch, 2 batches/core):
  - fp16 operands for projection + energy matmuls (fp32 runs at 1/4 PE rate;
    fp16 streams at bf16 rate with 8x the mantissa of bf16).
  - Stacked QK projection: one GEMM emits qkall[:, h] = [Q_h; K_h] (stacked
    weight matrix, host-prepped), so the energy's Z operand is a direct
    slice - no per-head assembly copies.
  - E^T = Z^T U with Z = [Q_h; K_h], U = [P_h; Q_h] (row order within the
    stack is irrelevant as long as rows pair K<->Q and Q<->P). U lives in a
    static tile: pos half preloaded once, Q half refreshed per (b, h) by one
    SBUF->SBUF DMA (the partition shift DMA is off the compute engines).
  - exp without max-subtraction (logits bounded, safe in fp32 PSUM),
    T^T = exp(E^T) stored bf16 (range needs bf16, fp16 would overflow).
  - denominator via ones-augmented V^T (padded layout, 65 cols per head with
    the 65th = 1.0), AV matmuls bf16.
  - normalize: den broadcast via rank-1 matmul, reciprocal_approx_fast
    (single-pass custom DVE op - the stock 8-cycle/elem reciprocal was the
    pipeline's serial bottleneck), DVE multiply, GpSimd residual add.
"""

import sys

import numpy as np

try:
    import concourse.bass as bass  # noqa: F401
except Exception:  # pragma: no cover
    sys.path.insert(0, "/opt/trn_rl_repo")

import concourse.bass as bass  # noqa: F401
import concourse.tile as tile
from concourse import bacc, mybir
from concourse.bass_utils import run_bass_kernel_spmd

F32 = mybir.dt.float32
F16 = mybir.dt.float16
BF16 = mybir.dt.bfloat16
EXP = mybir.ActivationFunctionType.Exp

N_CORES = 8
B, C, WD, HD = 16, 512, 32, 32
HEAD, D, N = 8, 64, 1024
BPC = B // N_CORES  # batches per core


def build_bass():
    nc = bacc.Bacc("TRN2")

    xs_d = nc.dram_tensor("xs", [BPC, C, N], F32, kind="ExternalInput")
    xh_d = nc.dram_tensor("xh", [BPC, C, N], F16, kind="ExternalInput")
    wqkt_d = nc.dram_tensor("wqkt", [4, 128, 1024], F16, kind="ExternalInput")
    wvt_d = nc.dram_tensor("wvt", [4, 128, 512], F16, kind="ExternalInput")
    bqk_d = nc.dram_tensor("bqk", [8, 128, 1], F32, kind="ExternalInput")
    pos_d = nc.dram_tensor("pos", [HEAD, D, N], F16, kind="ExternalInput")
    out_d = nc.dram_tensor("out", [BPC, C, N], F32, kind="ExternalOutput")

    with tile.TileContext(nc) as tc:
        with (
            tc.tile_pool(name="consts", bufs=1) as cpool,
            tc.tile_pool(name="work", bufs=2) as wpool,
            tc.tile_pool(name="psume", bufs=2, space="PSUM") as pse,
            tc.tile_pool(name="psumo", bufs=2, space="PSUM") as pso,
        ):
            # ---- batch-0 X + weights, emission-interleaved so the first
            # projection's operands land first in the DMA queues ----
            wqkt_sb = cpool.tile([128, 4, 1024], F16, name="wqkt_sb")
            wvt_sb = cpool.tile([128, 4, 512], F16, name="wvt_sb")
            bqk_sb = cpool.tile([128, 8, 1], F32, name="bqk_sb")

            def prep_x(b, wq_too=False):
                x_sb = wpool.tile([128, 4, N], F16, name=f"x_{b}", tag="x")
                for kc in range(4):
                    if wq_too:
                        nc.sync.dma_start(wqkt_sb[:, kc, :], wqkt_d[kc])
                    nc.sync.dma_start(x_sb[:, kc, :], xh_d[b, kc * 128:(kc + 1) * 128, :])
                return x_sb

            ctx = {0: {}}
            ctx[0]["x"] = prep_x(0, wq_too=True)
            for h in range(8):
                nc.sync.dma_start(bqk_sb[:, h, :], bqk_d[h])

            # static per-head U tiles: pos in partitions 0-63 (loaded once),
            # Q_h refreshed into partitions 64-127 each (b, h).
            u_all = cpool.tile([128, 8, N], F16, name="u_all")
            nc.sync.dma_start(u_all[0:64, 0, :], pos_d[0])
            for kc in range(4):
                nc.sync.dma_start(wvt_sb[:, kc, :], wvt_d[kc])
            for h in range(1, 8):
                nc.sync.dma_start(u_all[0:64, h, :], pos_d[h])
            onesbf = cpool.tile([65, 64], BF16, name="onesbf")
            nc.vector.memset(onesbf[:], 1.0)
            zbias = cpool.tile([128, 1], F32, name="zbias")
            nc.vector.memset(zbias[:], 0.0)

            def emit_qkproj_slot(b, h):
                # stacked QK projection slot h: qkall[:, h] = [Q_h; K_h]
                # (f16, bias added): 2 psum groups of 4 accumulating matmuls.
                cx = ctx[b]
                if "qkall" not in cx:
                    cx["qkall"] = wpool.tile(
                        [128, 8, N], F16, name=f"qkall_{b}", tag="qkall", bufs=2
                    )
                qkall = cx["qkall"]
                for nh in range(2):
                    ps = pso.tile([128, 512], F32, name=f"ps_p{b}{h}{nh}", tag="pso")
                    for kc in range(4):
                        nc.tensor.matmul(
                            ps[:],
                            wqkt_sb[:, kc, h * 128:(h + 1) * 128],
                            cx["x"][:, kc, nh * 512:(nh + 1) * 512],
                            start=(kc == 0),
                            stop=(kc == 3),
                        )
                    nc.vector.tensor_scalar_add(
                        qkall[:, h, nh * 512:(nh + 1) * 512], ps[:], bqk_sb[:, h, :]
                    )

            def emit_u(b, h):
                # U_h bottom half <- Q_h (partitions 0-63 of qkall slot h ->
                # partitions 64-127 of u_all slot h): partition shift, so DMA.
                nc.sync.dma_start(u_all[64:128, h, :], ctx[b]["qkall"][0:64, h, :])

            def emit_vproj(b, c0, c1):
                # V^T projection, bf16 out, weights only. The V bias is
                # folded into the residual host-side (num = rawAV + bv*den,
                # so out = rawAV/den + bv); the 65th ones-column per head
                # (softmax denominator) is memset once per vpt tile.
                cx = ctx[b]
                if "vpt" not in cx:
                    cx["vpt"] = wpool.tile([128, 8, 8, 65], BF16, name=f"vpt_{b}", tag="vpt")
                    nc.vector.memset(cx["vpt"][:, :, :, 64], 1.0)
                vpt = cx["vpt"]
                for nc8 in range(c0, c1):
                    ps = pso.tile([128, 8, 64], F32, name=f"ps_v{b}{nc8}", tag="pso")
                    for kc in range(4):
                        nc.tensor.matmul(
                            ps[:, :, :],
                            cx["x"][:, kc, nc8 * 128:(nc8 + 1) * 128],
                            wvt_sb[:, kc, :],
                            start=(kc == 0),
                            stop=(kc == 3),
                        )
                    nc.vector.tensor_copy(vpt[:, nc8, :, 0:64], ps[:, :, :])

            def emit_head(b, h, vproj_chunks=False):
                # Fused energy/exp/AV pipeline over the 8 m-chunks: for each
                # chunk j, E^T matmuls -> exp -> immediately accumulate the
                # AV matmuls (contraction over m is the j loop). Z = qkall
                # slot h (direct slice), U = u_all slot h (pos + DMA'd Q).
                cx = ctx[b]
                qkall = cx["qkall"]
                ops = pso.tile([65, N], F32, name=f"ps_o{b}{h}", tag="pso")
                for j in range(8):
                    if vproj_chunks:
                        emit_vproj(b, j, j + 1)
                    eps = pse.tile([128, N], F32, name=f"ps_e{b}{h}{j}", tag="pse")
                    for ih in range(2):
                        nc.tensor.matmul(
                            eps[:, ih * 512:(ih + 1) * 512],
                            qkall[:, h, j * 128:(j + 1) * 128],
                            u_all[:, h, ih * 512:(ih + 1) * 512],
                            start=True,
                            stop=True,
                        )
                    tt = wpool.tile([128, N], BF16, name=f"tt_{b}_{h}_{j}", tag="tt", bufs=8)
                    nc.scalar.activation(tt[:], eps[:], EXP, bias=zbias[:])
                    vpt = cx["vpt"]
                    for mh in range(2):
                        nc.tensor.matmul(
                            ops[:, mh * 512:(mh + 1) * 512],
                            vpt[:, j, h, :],
                            tt[:, mh * 512:(mh + 1) * 512],
                            start=(j == 0),
                            stop=(j == 7),
                        )
                # normalize: den -> bf16 row, rank-1 broadcast matmul,
                # single-pass approx reciprocal, multiply, residual-add on
                # GpSimd, store fp32.
                dhi = wpool.tile([65, N], BF16, name=f"dhi_{b}_{h}", tag="dhi")
                nc.vector.tensor_copy(dhi[64:65, :], ops[64:65, :])
                rps = pse.tile([64, N], F32, name=f"ps_r{b}{h}", tag="pse")
                for mh in range(2):
                    nc.tensor.matmul(
                        rps[:, mh * 512:(mh + 1) * 512],
                        onesbf[64:65, 0:64],
                        dhi[64:65, mh * 512:(mh + 1) * 512],
                        start=True,
                        stop=True,
                    )
                rbinv = wpool.tile([64, N], F32, name=f"rbinv_{b}_{h}", tag="rbinv")
                nc.vector.reciprocal_approx_fast(out=rbinv[:], in_=rps[:])
                osb = wpool.tile([64, N], F32, name=f"osb_{b}_{h}", tag="ostage", bufs=3)
                nc.vector.tensor_mul(osb[:], ops[0:64, :], rbinv[:])
                xres = wpool.tile([64, N], F32, name=f"xres_{b}_{h}", tag="xres")
                nc.sync.dma_start(xres[:], xs_d[b, h * 64:(h + 1) * 64, :])
                fin = wpool.tile([64, N], F32, name=f"fin_{b}_{h}", tag="ostage", bufs=3)
                nc.gpsimd.tensor_add(fin[:], osb[:], xres[:])
                nc.sync.dma_start(out_d[b, h * 64:(h + 1) * 64, :], fin[:])

            # ---- software pipeline over (batch, head): per head the
            # energy/exp/AV chunks are fused; the next head's projection
            # slot + U refresh are emitted at the end of each head so their
            # PE work fills the ACT-bound gaps; next batch's X load +
            # projections are emitted mid-way through the previous batch ----
            done = set()

            def proj_slot_once(b, h):
                if ("p", b, h) not in done:
                    done.add(("p", b, h))
                    emit_qkproj_slot(b, h)

            def u_once(b, h):
                if ("u", b, h) not in done:
                    done.add(("u", b, h))
                    emit_u(b, h)

            proj_slot_once(0, 0)
            u_once(0, 0)
            for b in range(BPC):
                for h in range(8):
                    emit_head(b, h, vproj_chunks=(b == 0 and h == 0))
                    if h < 7:
                        proj_slot_once(b, h + 1)
                        u_once(b, h + 1)
                    if b + 1 < BPC:
                        if h == 4:
                            ctx[b + 1] = {"x": prep_x(b + 1)}
                            for hh in range(8):
                                proj_slot_once(b + 1, hh)
                            # only heads whose energy(b, hh) is already
                            # emitted may have their U slot overwritten --
                            # Tile orders by emission, so emitting early
                            # would invert the WAR dependency.
                            for hh in range(5):
                                u_once(b + 1, hh)
                        elif h in (5, 6, 7):
                            u_once(b + 1, h)
                            emit_vproj(b + 1, 3 * (h - 5), min(8, 3 * (h - 5) + 3))

    nc.compile()
    return nc


def _prep_consts(Wq, bq, Wk, bk, Wv, bv, rel_h, rel_w):
    # stacked QK weight: output slot h covers channels [Wq head h; Wk head h]
    wstack = np.empty((1024, 512), np.float32)
    bstack = np.empty((8, 128, 1), np.float32)
    for h in range(HEAD):
        wstack[h * 128:h * 128 + 64] = Wq[h * 64:(h + 1) * 64]
        wstack[h * 128 + 64:h * 128 + 128] = Wk[h * 64:(h + 1) * 64]
        bstack[h, 0:64, 0] = bq[h * 64:(h + 1) * 64]
        bstack[h, 64:128, 0] = bk[h * 64:(h + 1) * 64]
    wqkt = np.ascontiguousarray(wstack.T).reshape(4, 128, 1024).astype(np.float16)
    wvt = np.ascontiguousarray(Wv.T).reshape(4, 128, 512).astype(np.float16)
    pos = (rel_h + rel_w).reshape(HEAD, D, N).astype(np.float16)
    return {
        "wqkt": wqkt,
        "wvt": wvt,
        "bqk": bstack,
        "pos": pos,
    }


_CACHE = {}


def build_in_maps(x, Wq, bq, Wk, bk, Wv, bv, rel_h, rel_w):
    x = np.asarray(x, np.float32)
    consts = _prep_consts(
        *[np.asarray(a, np.float32) for a in (Wq, bq, Wk, bk, Wv, bv, rel_h, rel_w)]
    )
    xr = x.reshape(B, C, N)
    xh = xr.astype(np.float16)
    # V bias folded into the residual: out = rawAV/den + bv + x
    xres = xr + np.asarray(bv, np.float32)[None, :, None]
    in_maps = []
    for c in range(N_CORES):
        m = dict(consts)
        m["xs"] = np.ascontiguousarray(xres[c * BPC:(c + 1) * BPC])
        m["xh"] = np.ascontiguousarray(xh[c * BPC:(c + 1) * BPC])
        in_maps.append(m)
    return in_maps


def kernel(x, Wq, bq, Wk, bk, Wv, bv, rel_h, rel_w, reg_qk, reg_v):
    # reg_qk / reg_v are computed-then-dropped by the reference -> unused.
    in_maps = build_in_maps(x, Wq, bq, Wk, bk, Wv, bv, rel_h, rel_w)

    if "nc" not in _CACHE:
        _CACHE["nc"] = build_bass()
    res = run_bass_kernel_spmd(_CACHE["nc"], in_maps, list(range(N_CORES)))
    outs = [np.asarray(r["out"]) for r in res.results]
    return np.concatenate(outs, axis=0).reshape(B, C, WD, HD)


if __name__ == "__main__":
    nc = build_bass()
    print("built ok")


# revision 16
# speedup vs baseline: 1.1229x; 1.1229x over previous
"""Trainium2 Bass kernel for nn_MHSA_40346922778634.

Math (per batch b, head h; the reference computes-then-drops the register
group, so reg_qk/reg_v are dead inputs):
  X = x[b] as [C=512, N=1024]
  Q = Wq X + bq ; K = Wk X + bk ; V = Wv X + bv       (per head: [64, N])
  P_h = (rel_h + rel_w) reshaped [head, 64, N]
  E[i,j] = Q_h[:,i].K_h[:,j] + P_h[:,i].Q_h[:,j]      ([N, N])
  attn = softmax(E, axis=-1)
  Out_h = V_h @ attn^T ; out[b, h*64:(h+1)*64] = Out_h + X[h*64:(h+1)*64]

Kernel strategy (8 cores, data-parallel over batch, 2 batches/core):
  - fp16 operands for projection + energy matmuls (fp32 runs at 1/4 PE rate;
    fp16 streams at bf16 rate with 8x the mantissa of bf16).
  - Stacked QK projection: one GEMM emits qkall[:, h] = [Q_h; K_h] (stacked
    weight matrix, host-prepped), so the energy's Z operand is a direct
    slice - no per-head assembly copies.
  - E^T = Z^T U with Z = [Q_h; K_h], U = [P_h; Q_h] (row order within the
    stack is irrelevant as long as rows pair K<->Q and Q<->P). U lives in a
    static tile: pos half preloaded once, Q half refreshed per (b, h) by one
    SBUF->SBUF DMA (the partition shift DMA is off the compute engines).
  - exp without max-subtraction (logits bounded, safe in fp32 PSUM),
    T^T = exp(E^T) stored bf16 (range needs bf16, fp16 would overflow).
  - denominator via ones-augmented V^T (padded layout, 65 cols per head with
    the 65th = 1.0), AV matmuls bf16.
  - normalize: den broadcast via rank-1 matmul, reciprocal_approx_fast
    (single-pass custom DVE op - the stock 8-cycle/elem reciprocal was the
    pipeline's serial bottleneck), DVE multiply, GpSimd residual add.
"""

import sys

import numpy as np

try:
    import concourse.bass as bass  # noqa: F401
except Exception:  # pragma: no cover
    sys.path.insert(0, "/opt/trn_rl_repo")

import concourse.bass as bass  # noqa: F401
import concourse.tile as tile
from concourse import bacc, mybir
from concourse.bass_utils import run_bass_kernel_spmd

F32 = mybir.dt.float32
F16 = mybir.dt.float16
BF16 = mybir.dt.bfloat16
EXP = mybir.ActivationFunctionType.Exp

N_CORES = 8
B, C, WD, HD = 16, 512, 32, 32
HEAD, D, N = 8, 64, 1024
BPC = B // N_CORES  # batches per core


def build_bass():
    nc = bacc.Bacc("TRN2")

    xs_d = nc.dram_tensor("xs", [BPC, C, N], F32, kind="ExternalInput")
    xh_d = nc.dram_tensor("xh", [BPC, C, N], F16, kind="ExternalInput")
    wqkt_d = nc.dram_tensor("wqkt", [4, 128, 1024], F16, kind="ExternalInput")
    wvt_d = nc.dram_tensor("wvt", [4, 128, 512], F16, kind="ExternalInput")
    bqk_d = nc.dram_tensor("bqk", [8, 128, 1], F32, kind="ExternalInput")
    pos_d = nc.dram_tensor("pos", [HEAD, D, N], F16, kind="ExternalInput")
    out_d = nc.dram_tensor("out", [BPC, C, N], F32, kind="ExternalOutput")

    with tile.TileContext(nc) as tc:
        with (
            tc.tile_pool(name="consts", bufs=1) as cpool,
            tc.tile_pool(name="work", bufs=2) as wpool,
            tc.tile_pool(name="psume", bufs=2, space="PSUM") as pse,
            tc.tile_pool(name="psumo", bufs=2, space="PSUM") as pso,
        ):
            # ---- batch-0 X + weights, emission-interleaved so the first
            # projection's operands land first in the DMA queues ----
            wqkt_sb = cpool.tile([128, 4, 1024], F16, name="wqkt_sb")
            wvt_sb = cpool.tile([128, 4, 512], F16, name="wvt_sb")
            bqk_sb = cpool.tile([128, 8, 1], F32, name="bqk_sb")

            def prep_x(b, wq_too=False):
                x_sb = wpool.tile([128, 4, N], F16, name=f"x_{b}", tag="x")
                for kc in range(4):
                    if wq_too:
                        nc.sync.dma_start(wqkt_sb[:, kc, :], wqkt_d[kc])
                    nc.sync.dma_start(x_sb[:, kc, :], xh_d[b, kc * 128:(kc + 1) * 128, :])
                return x_sb

            ctx = {0: {}}
            ctx[0]["x"] = prep_x(0, wq_too=True)
            for h in range(8):
                nc.sync.dma_start(bqk_sb[:, h, :], bqk_d[h])

            # static per-head U tiles: pos in partitions 0-63 (loaded once),
            # Q_h refreshed into partitions 64-127 each (b, h).
            u_all = cpool.tile([128, 8, N], F16, name="u_all")
            nc.sync.dma_start(u_all[0:64, 0, :], pos_d[0])
            for kc in range(4):
                nc.sync.dma_start(wvt_sb[:, kc, :], wvt_d[kc])
            for h in range(1, 8):
                nc.sync.dma_start(u_all[0:64, h, :], pos_d[h])
            onesbf = cpool.tile([65, 64], BF16, name="onesbf")
            nc.vector.memset(onesbf[:], 1.0)
            zbias = cpool.tile([128, 1], F32, name="zbias")
            nc.vector.memset(zbias[:], 0.0)

            def emit_qkproj_slot(b, h):
                # stacked QK projection slot h: qkall[:, h] = [Q_h; K_h]
                # (f16, bias added): 2 psum groups of 4 accumulating matmuls.
                cx = ctx[b]
                if "qkall" not in cx:
                    cx["qkall"] = wpool.tile(
                        [128, 8, N], F16, name=f"qkall_{b}", tag="qkall", bufs=2
                    )
                qkall = cx["qkall"]
                for nh in range(2):
                    ps = pso.tile([128, 512], F32, name=f"ps_p{b}{h}{nh}", tag="pso")
                    for kc in range(4):
                        nc.tensor.matmul(
                            ps[:],
                            wqkt_sb[:, kc, h * 128:(h + 1) * 128],
                            cx["x"][:, kc, nh * 512:(nh + 1) * 512],
                            start=(kc == 0),
                            stop=(kc == 3),
                        )
                    nc.vector.tensor_scalar_add(
                        qkall[:, h, nh * 512:(nh + 1) * 512], ps[:], bqk_sb[:, h, :]
                    )

            def emit_u(b, h):
                # U_h bottom half <- Q_h (partitions 0-63 of qkall slot h ->
                # partitions 64-127 of u_all slot h): partition shift, so DMA.
                nc.sync.dma_start(u_all[64:128, h, :], ctx[b]["qkall"][0:64, h, :])

            def emit_vproj(b, c0, c1):
                # V^T projection, bf16 out, weights only. The V bias is
                # folded into the residual host-side (num = rawAV + bv*den,
                # so out = rawAV/den + bv); the 65th ones-column per head
                # (softmax denominator) is memset once per vpt tile.
                cx = ctx[b]
                if "vpt" not in cx:
                    cx["vpt"] = wpool.tile([128, 8, 8, 65], BF16, name=f"vpt_{b}", tag="vpt")
                    nc.vector.memset(cx["vpt"][:, :, :, 64], 1.0)
                vpt = cx["vpt"]
                for nc8 in range(c0, c1):
                    ps = pso.tile([128, 8, 64], F32, name=f"ps_v{b}{nc8}", tag="pso")
                    for kc in range(4):
                        nc.tensor.matmul(
                            ps[:, :, :],
                            cx["x"][:, kc, nc8 * 128:(nc8 + 1) * 128],
                            wvt_sb[:, kc, :],
                            start=(kc == 0),
                            stop=(kc == 3),
                        )
                    nc.vector.tensor_copy(vpt[:, nc8, :, 0:64], ps[:, :, :])

            tt_of = {}

            def emit_E_chunk(b, h, j):
                # E^T chunk j + exp. Z = qkall slot h (direct slice),
                # U = u_all slot h (pos + DMA'd Q half).
                qkall = ctx[b]["qkall"]
                eps = pse.tile([128, N], F32, name=f"ps_e{b}{h}{j}", tag="pse")
                for ih in range(2):
                    nc.tensor.matmul(
                        eps[:, ih * 512:(ih + 1) * 512],
                        qkall[:, h, j * 128:(j + 1) * 128],
                        u_all[:, h, ih * 512:(ih + 1) * 512],
                        start=True,
                        stop=True,
                    )
                tt = wpool.tile([128, N], BF16, name=f"tt_{b}_{h}_{j}", tag="tt", bufs=8)
                nc.scalar.activation(tt[:], eps[:], EXP, bias=zbias[:])
                tt_of[(b, h, j)] = tt

            ops_of = {}

            def emit_AV_chunk(b, h, j):
                # AV accumulation for m-chunk j. Emitted LAG global chunks
                # behind the E/exp emission: the eps double-buffer already
                # forces exp(ci-2) done before E(ci) issues, so by the time
                # the PE's strict-FIFO queue reaches this matmul its tt is
                # ready and the engine never stalls on ACT.
                cx = ctx[b]
                if j == 0:
                    ops_of[(b, h)] = pso.tile([65, N], F32, name=f"ps_o{b}{h}", tag="pso")
                ops = ops_of[(b, h)]
                tt = tt_of.pop((b, h, j))
                for mh in range(2):
                    nc.tensor.matmul(
                        ops[:, mh * 512:(mh + 1) * 512],
                        cx["vpt"][:, j, h, :],
                        tt[:, mh * 512:(mh + 1) * 512],
                        start=(j == 0),
                        stop=(j == 7),
                    )

            def emit_norm(b, h):
                # normalize: den -> bf16 row, rank-1 broadcast matmul,
                # single-pass approx reciprocal, multiply, residual-add on
                # GpSimd, store fp32.
                ops = ops_of.pop((b, h))
                dhi = wpool.tile([65, N], BF16, name=f"dhi_{b}_{h}", tag="dhi")
                nc.vector.tensor_copy(dhi[64:65, :], ops[64:65, :])
                rps = pse.tile([64, N], F32, name=f"ps_r{b}{h}", tag="pse")
                for mh in range(2):
                    nc.tensor.matmul(
                        rps[:, mh * 512:(mh + 1) * 512],
                        onesbf[64:65, 0:64],
                        dhi[64:65, mh * 512:(mh + 1) * 512],
                        start=True,
                        stop=True,
                    )
                rbinv = wpool.tile([64, N], F32, name=f"rbinv_{b}_{h}", tag="rbinv")
                nc.vector.reciprocal_approx_fast(out=rbinv[:], in_=rps[:])
                osb = wpool.tile([64, N], F32, name=f"osb_{b}_{h}", tag="ostage", bufs=3)
                nc.vector.tensor_mul(osb[:], ops[0:64, :], rbinv[:])
                xres = wpool.tile([64, N], F32, name=f"xres_{b}_{h}", tag="xres")
                nc.sync.dma_start(xres[:], xs_d[b, h * 64:(h + 1) * 64, :])
                fin = wpool.tile([64, N], F32, name=f"fin_{b}_{h}", tag="ostage", bufs=3)
                nc.gpsimd.tensor_add(fin[:], osb[:], xres[:])
                nc.sync.dma_start(out_d[b, h * 64:(h + 1) * 64, :], fin[:])

            # ---- software pipeline over (batch, head): per head the
            # energy/exp/AV chunks are fused; the next head's projection
            # slot + U refresh are emitted at the end of each head so their
            # PE work fills the ACT-bound gaps; next batch's X load +
            # projections are emitted mid-way through the previous batch ----
            done = set()

            def proj_slot_once(b, h):
                if ("p", b, h) not in done:
                    done.add(("p", b, h))
                    emit_qkproj_slot(b, h)

            def u_once(b, h):
                if ("u", b, h) not in done:
                    done.add(("u", b, h))
                    emit_u(b, h)

            proj_slot_once(0, 0)
            u_once(0, 0)
            LAG = 4  # AV chunks trail E/exp chunks by this many global chunks
            av_pend = []

            def pop_av():
                pb, ph, pj = av_pend.pop(0)
                emit_AV_chunk(pb, ph, pj)
                if pj == 7:
                    emit_norm(pb, ph)

            for b in range(BPC):
                for h in range(8):
                    for j in range(8):
                        if b == 0 and h == 0:
                            emit_vproj(0, j, j + 1)
                        emit_E_chunk(b, h, j)
                        av_pend.append((b, h, j))
                        if len(av_pend) > LAG:
                            pop_av()
                    if h < 7:
                        proj_slot_once(b, h + 1)
                        u_once(b, h + 1)
                    if b + 1 < BPC:
                        if h == 4:
                            ctx[b + 1] = {"x": prep_x(b + 1)}
                            for hh in range(8):
                                proj_slot_once(b + 1, hh)
                            # only heads whose energy(b, hh) is already
                            # emitted may have their U slot overwritten --
                            # Tile orders by emission, so emitting early
                            # would invert the WAR dependency.
                            for hh in range(5):
                                u_once(b + 1, hh)
                        elif h in (5, 6, 7):
                            u_once(b + 1, h)
                            emit_vproj(b + 1, 3 * (h - 5), min(8, 3 * (h - 5) + 3))
            while av_pend:
                pop_av()

    nc.compile()
    return nc


def _prep_consts(Wq, bq, Wk, bk, Wv, bv, rel_h, rel_w):
    # stacked QK weight: output slot h covers channels [Wq head h; Wk head h]
    wstack = np.empty((1024, 512), np.float32)
    bstack = np.empty((8, 128, 1), np.float32)
    for h in range(HEAD):
        wstack[h * 128:h * 128 + 64] = Wq[h * 64:(h + 1) * 64]
        wstack[h * 128 + 64:h * 128 + 128] = Wk[h * 64:(h + 1) * 64]
        bstack[h, 0:64, 0] = bq[h * 64:(h + 1) * 64]
        bstack[h, 64:128, 0] = bk[h * 64:(h + 1) * 64]
    wqkt = np.ascontiguousarray(wstack.T).reshape(4, 128, 1024).astype(np.float16)
    wvt = np.ascontiguousarray(Wv.T).reshape(4, 128, 512).astype(np.float16)
    pos = (rel_h + rel_w).reshape(HEAD, D, N).astype(np.float16)
    return {
        "wqkt": wqkt,
        "wvt": wvt,
        "bqk": bstack,
        "pos": pos,
    }


_CACHE = {}


def build_in_maps(x, Wq, bq, Wk, bk, Wv, bv, rel_h, rel_w):
    x = np.asarray(x, np.float32)
    consts = _prep_consts(
        *[np.asarray(a, np.float32) for a in (Wq, bq, Wk, bk, Wv, bv, rel_h, rel_w)]
    )
    xr = x.reshape(B, C, N)
    xh = xr.astype(np.float16)
    # V bias folded into the residual: out = rawAV/den + bv + x
    xres = xr + np.asarray(bv, np.float32)[None, :, None]
    in_maps = []
    for c in range(N_CORES):
        m = dict(consts)
        m["xs"] = np.ascontiguousarray(xres[c * BPC:(c + 1) * BPC])
        m["xh"] = np.ascontiguousarray(xh[c * BPC:(c + 1) * BPC])
        in_maps.append(m)
    return in_maps


def kernel(x, Wq, bq, Wk, bk, Wv, bv, rel_h, rel_w, reg_qk, reg_v):
    # reg_qk / reg_v are computed-then-dropped by the reference -> unused.
    in_maps = build_in_maps(x, Wq, bq, Wk, bk, Wv, bv, rel_h, rel_w)

    if "nc" not in _CACHE:
        _CACHE["nc"] = build_bass()
    res = run_bass_kernel_spmd(_CACHE["nc"], in_maps, list(range(N_CORES)))
    outs = [np.asarray(r["out"]) for r in res.results]
    return np.concatenate(outs, axis=0).reshape(B, C, WD, HD)


if __name__ == "__main__":
    nc = build_bass()
    print("built ok")


# revision 17
# speedup vs baseline: 1.3124x; 1.1688x over previous
"""Trainium2 Bass kernel for nn_MHSA_40346922778634.

Math (per batch b, head h; the reference computes-then-drops the register
group, so reg_qk/reg_v are dead inputs):
  X = x[b] as [C=512, N=1024]
  Q = Wq X + bq ; K = Wk X + bk ; V = Wv X + bv       (per head: [64, N])
  P_h = (rel_h + rel_w) reshaped [head, 64, N]
  E[i,j] = Q_h[:,i].K_h[:,j] + P_h[:,i].Q_h[:,j]      ([N, N])
  attn = softmax(E, axis=-1)
  Out_h = V_h @ attn^T ; out[b, h*64:(h+1)*64] = Out_h + X[h*64:(h+1)*64]

Kernel strategy (8 cores, data-parallel over batch, 2 batches/core):
  - fp16 operands for projection + energy matmuls (fp32 runs at 1/4 PE rate;
    fp16 streams at bf16 rate with 8x the mantissa of bf16).
  - Stacked QK projection: one GEMM emits qkall[:, h] = [Q_h; K_h] (stacked
    weight matrix, host-prepped), so the energy's Z operand is a direct
    slice - no per-head assembly copies.
  - E^T = Z^T U with Z = [Q_h; K_h], U = [P_h; Q_h] (row order within the
    stack is irrelevant as long as rows pair K<->Q and Q<->P). U lives in a
    static tile: pos half preloaded once, Q half refreshed per (b, h) by one
    SBUF->SBUF DMA (the partition shift DMA is off the compute engines).
  - exp without max-subtraction (logits bounded, safe in fp32 PSUM),
    T^T = exp(E^T) stored bf16 (range needs bf16, fp16 would overflow).
  - denominator via ones-augmented V^T (padded layout, 65 cols per head with
    the 65th = 1.0), AV matmuls bf16.
  - normalize: den broadcast via rank-1 matmul, reciprocal_approx_fast
    (single-pass custom DVE op - the stock 8-cycle/elem reciprocal was the
    pipeline's serial bottleneck), DVE multiply, GpSimd residual add.
"""

import sys

import numpy as np

try:
    import concourse.bass as bass  # noqa: F401
except Exception:  # pragma: no cover
    sys.path.insert(0, "/opt/trn_rl_repo")

import concourse.bass as bass  # noqa: F401
import concourse.tile as tile
from concourse import bacc, mybir
from concourse.bass_utils import run_bass_kernel_spmd

F32 = mybir.dt.float32
F16 = mybir.dt.float16
BF16 = mybir.dt.bfloat16
EXP = mybir.ActivationFunctionType.Exp

N_CORES = 8
B, C, WD, HD = 16, 512, 32, 32
HEAD, D, N = 8, 64, 1024
BPC = B // N_CORES  # batches per core


def build_bass():
    nc = bacc.Bacc("TRN2")

    xs_d = nc.dram_tensor("xs", [BPC, C, N], F32, kind="ExternalInput")
    xh_d = nc.dram_tensor("xh", [BPC, C, N], F16, kind="ExternalInput")
    wqkt_d = nc.dram_tensor("wqkt", [4, 128, 1024], F16, kind="ExternalInput")
    wvt_d = nc.dram_tensor("wvt", [4, 128, 512], F16, kind="ExternalInput")
    bqk_d = nc.dram_tensor("bqk", [8, 128, 1], F32, kind="ExternalInput")
    pos_d = nc.dram_tensor("pos", [HEAD, D, N], F16, kind="ExternalInput")
    out_d = nc.dram_tensor("out", [BPC, C, N], F32, kind="ExternalOutput")

    with tile.TileContext(nc) as tc:
        with (
            tc.tile_pool(name="consts", bufs=1) as cpool,
            tc.tile_pool(name="work", bufs=2) as wpool,
            tc.tile_pool(name="psume", bufs=2, space="PSUM") as pse,
            tc.tile_pool(name="psumo", bufs=2, space="PSUM") as pso,
        ):
            # ---- batch-0 X + weights, emission-interleaved so the first
            # projection's operands land first in the DMA queues ----
            wqkt_sb = cpool.tile([128, 4, 1024], F16, name="wqkt_sb")
            wvt_sb = cpool.tile([128, 4, 512], F16, name="wvt_sb")
            bqk_sb = cpool.tile([128, 8, 1], F32, name="bqk_sb")

            def prep_x(b, wq_too=False):
                x_sb = wpool.tile([128, 4, N], F16, name=f"x_{b}", tag="x")
                for kc in range(4):
                    if wq_too:
                        nc.sync.dma_start(wqkt_sb[:, kc, :], wqkt_d[kc])
                    nc.sync.dma_start(x_sb[:, kc, :], xh_d[b, kc * 128:(kc + 1) * 128, :])
                return x_sb

            ctx = {0: {}}
            ctx[0]["x"] = prep_x(0, wq_too=True)
            for h in range(8):
                nc.sync.dma_start(bqk_sb[:, h, :], bqk_d[h])

            # static per-head U tiles: pos in partitions 0-63 (loaded once),
            # Q_h refreshed into partitions 64-127 each (b, h).
            u_all = cpool.tile([128, 8, N], F16, name="u_all")
            nc.sync.dma_start(u_all[0:64, 0, :], pos_d[0])
            for kc in range(4):
                nc.sync.dma_start(wvt_sb[:, kc, :], wvt_d[kc])
            for h in range(1, 8):
                nc.sync.dma_start(u_all[0:64, h, :], pos_d[h])
            onesbf = cpool.tile([65, 64], BF16, name="onesbf")
            nc.vector.memset(onesbf[:], 1.0)
            zbias = cpool.tile([128, 1], F32, name="zbias")
            nc.vector.memset(zbias[:], 0.0)

            def emit_qkproj_slot(b, h):
                # stacked QK projection slot h: qkall[:, h] = [Q_h; K_h]
                # (f16, bias added): 2 psum groups of 4 accumulating matmuls.
                cx = ctx[b]
                if "qkall" not in cx:
                    cx["qkall"] = wpool.tile(
                        [128, 8, N], F16, name=f"qkall_{b}", tag="qkall", bufs=2
                    )
                qkall = cx["qkall"]
                for nh in range(2):
                    ps = pso.tile([128, 512], F32, name=f"ps_p{b}{h}{nh}", tag="pso")
                    for kc in range(4):
                        nc.tensor.matmul(
                            ps[:],
                            wqkt_sb[:, kc, h * 128:(h + 1) * 128],
                            cx["x"][:, kc, nh * 512:(nh + 1) * 512],
                            start=(kc == 0),
                            stop=(kc == 3),
                        )
                    nc.vector.tensor_scalar_add(
                        qkall[:, h, nh * 512:(nh + 1) * 512], ps[:], bqk_sb[:, h, :]
                    )

            def emit_u(b, h):
                # U_h bottom half <- Q_h (partitions 0-63 of qkall slot h ->
                # partitions 64-127 of u_all slot h): partition shift, so DMA.
                nc.sync.dma_start(u_all[64:128, h, :], ctx[b]["qkall"][0:64, h, :])

            def emit_vproj(b, c0, c1):
                # V^T projection, bf16 out, weights only. The V bias is
                # folded into the residual host-side (num = rawAV + bv*den,
                # so out = rawAV/den + bv); the 65th ones-column per head
                # (softmax denominator) is memset once per vpt tile.
                cx = ctx[b]
                if "vpt" not in cx:
                    cx["vpt"] = wpool.tile([128, 8, 8, 65], BF16, name=f"vpt_{b}", tag="vpt")
                    nc.vector.memset(cx["vpt"][:, :, :, 64], 1.0)
                vpt = cx["vpt"]
                for nc8 in range(c0, c1):
                    ps = pso.tile([128, 8, 64], F32, name=f"ps_v{b}{nc8}", tag="pso")
                    for kc in range(4):
                        nc.tensor.matmul(
                            ps[:, :, :],
                            cx["x"][:, kc, nc8 * 128:(nc8 + 1) * 128],
                            wvt_sb[:, kc, :],
                            start=(kc == 0),
                            stop=(kc == 3),
                        )
                    nc.vector.tensor_copy(vpt[:, nc8, :, 0:64], ps[:, :, :])

            tt_of = {}

            def emit_E_chunk(b, h, j):
                # E^T chunk j + exp. Z = qkall slot h (direct slice),
                # U = u_all slot h (pos + DMA'd Q half).
                qkall = ctx[b]["qkall"]
                eps = pse.tile([128, N], F32, name=f"ps_e{b}{h}{j}", tag="pse")
                for ih in range(2):
                    nc.tensor.matmul(
                        eps[:, ih * 512:(ih + 1) * 512],
                        qkall[:, h, j * 128:(j + 1) * 128],
                        u_all[:, h, ih * 512:(ih + 1) * 512],
                        start=True,
                        stop=True,
                    )
                tt = wpool.tile([128, N], BF16, name=f"tt_{b}_{h}_{j}", tag="tt", bufs=8)
                nc.scalar.activation(tt[:], eps[:], EXP, bias=zbias[:])
                tt_of[(b, h, j)] = tt

            ops_of = {}

            def emit_AV_chunk(b, h, j):
                # AV accumulation for m-chunk j. Emitted LAG global chunks
                # behind the E/exp emission: the eps double-buffer already
                # forces exp(ci-2) done before E(ci) issues, so by the time
                # the PE's strict-FIFO queue reaches this matmul its tt is
                # ready and the engine never stalls on ACT.
                cx = ctx[b]
                if j == 0:
                    ops_of[(b, h)] = pso.tile([65, N], F32, name=f"ps_o{b}{h}", tag="pso")
                ops = ops_of[(b, h)]
                tt = tt_of.pop((b, h, j))
                for mh in range(2):
                    nc.tensor.matmul(
                        ops[:, mh * 512:(mh + 1) * 512],
                        cx["vpt"][:, j, h, :],
                        tt[:, mh * 512:(mh + 1) * 512],
                        start=(j == 0),
                        stop=(j == 7),
                    )

            def emit_norm(b, h):
                # normalize: den -> bf16 row, rank-1 broadcast matmul,
                # single-pass approx reciprocal, multiply, residual-add on
                # GpSimd, store fp32.
                ops = ops_of.pop((b, h))
                dhi = wpool.tile([65, N], BF16, name=f"dhi_{b}_{h}", tag="dhi")
                nc.vector.tensor_copy(dhi[64:65, :], ops[64:65, :])
                rps = pse.tile([64, N], F32, name=f"ps_r{b}{h}", tag="pse")
                for mh in range(2):
                    nc.tensor.matmul(
                        rps[:, mh * 512:(mh + 1) * 512],
                        onesbf[64:65, 0:64],
                        dhi[64:65, mh * 512:(mh + 1) * 512],
                        start=True,
                        stop=True,
                    )
                rbinv = wpool.tile([64, N], F32, name=f"rbinv_{b}_{h}", tag="rbinv")
                nc.vector.reciprocal_approx_fast(out=rbinv[:], in_=rps[:])
                osb = wpool.tile([64, N], F32, name=f"osb_{b}_{h}", tag="ostage", bufs=3)
                nc.vector.tensor_mul(osb[:], ops[0:64, :], rbinv[:])
                xres = wpool.tile([64, N], F32, name=f"xres_{b}_{h}", tag="xres")
                nc.sync.dma_start(xres[:], xs_d[b, h * 64:(h + 1) * 64, :])
                fin = wpool.tile([64, N], F32, name=f"fin_{b}_{h}", tag="ostage", bufs=3)
                nc.gpsimd.tensor_add(fin[:], osb[:], xres[:])
                nc.sync.dma_start(out_d[b, h * 64:(h + 1) * 64, :], fin[:])

            # ---- software pipeline over (batch, head): per head the
            # energy/exp/AV chunks are fused; the next head's projection
            # slot + U refresh are emitted at the end of each head so their
            # PE work fills the ACT-bound gaps; next batch's X load +
            # projections are emitted mid-way through the previous batch ----
            done = set()

            def proj_slot_once(b, h):
                if ("p", b, h) not in done:
                    done.add(("p", b, h))
                    emit_qkproj_slot(b, h)

            def u_once(b, h):
                if ("u", b, h) not in done:
                    done.add(("u", b, h))
                    emit_u(b, h)

            proj_slot_once(0, 0)
            u_once(0, 0)
            LAG = 4  # AV chunks trail E/exp chunks by this many global chunks
            av_pend = []

            def pop_av():
                pb, ph, pj = av_pend.pop(0)
                emit_AV_chunk(pb, ph, pj)
                if pj == 7:
                    emit_norm(pb, ph)

            emit_vproj(0, 0, 4)
            for b in range(BPC):
                for h in range(8):
                    for j in range(8):
                        if b == 0 and h == 0 and j >= 4:
                            emit_vproj(0, j, j + 1)
                        emit_E_chunk(b, h, j)
                        av_pend.append((b, h, j))
                        if len(av_pend) > LAG:
                            pop_av()
                        if j == 3 and h < 7:
                            # mid-head prefetch: next head's projection slot
                            # + U refresh, so the proj->bias->DMA chain
                            # completes before the next head's first energy
                            proj_slot_once(b, h + 1)
                            u_once(b, h + 1)
                    if b + 1 < BPC:
                        if h == 4:
                            ctx[b + 1] = {"x": prep_x(b + 1)}
                            for hh in range(8):
                                proj_slot_once(b + 1, hh)
                            # only heads whose energy(b, hh) is already
                            # emitted may have their U slot overwritten --
                            # Tile orders by emission, so emitting early
                            # would invert the WAR dependency.
                            for hh in range(5):
                                u_once(b + 1, hh)
                        elif h in (5, 6, 7):
                            u_once(b + 1, h)
                            emit_vproj(b + 1, 3 * (h - 5), min(8, 3 * (h - 5) + 3))
            while av_pend:
                pop_av()

    nc.compile()
    return nc


def _prep_consts(Wq, bq, Wk, bk, Wv, bv, rel_h, rel_w):
    # stacked QK weight: output slot h covers channels [Wq head h; Wk head h]
    wstack = np.empty((1024, 512), np.float32)
    bstack = np.empty((8, 128, 1), np.float32)
    for h in range(HEAD):
        wstack[h * 128:h * 128 + 64] = Wq[h * 64:(h + 1) * 64]
        wstack[h * 128 + 64:h * 128 + 128] = Wk[h * 64:(h + 1) * 64]
        bstack[h, 0:64, 0] = bq[h * 64:(h + 1) * 64]
        bstack[h, 64:128, 0] = bk[h * 64:(h + 1) * 64]
    wqkt = np.ascontiguousarray(wstack.T).reshape(4, 128, 1024).astype(np.float16)
    wvt = np.ascontiguousarray(Wv.T).reshape(4, 128, 512).astype(np.float16)
    pos = (rel_h + rel_w).reshape(HEAD, D, N).astype(np.float16)
    return {
        "wqkt": wqkt,
        "wvt": wvt,
        "bqk": bstack,
        "pos": pos,
    }


_CACHE = {}


def build_in_maps(x, Wq, bq, Wk, bk, Wv, bv, rel_h, rel_w):
    x = np.asarray(x, np.float32)
    consts = _prep_consts(
        *[np.asarray(a, np.float32) for a in (Wq, bq, Wk, bk, Wv, bv, rel_h, rel_w)]
    )
    xr = x.reshape(B, C, N)
    xh = xr.astype(np.float16)
    # V bias folded into the residual: out = rawAV/den + bv + x
    xres = xr + np.asarray(bv, np.float32)[None, :, None]
    in_maps = []
    for c in range(N_CORES):
        m = dict(consts)
        m["xs"] = np.ascontiguousarray(xres[c * BPC:(c + 1) * BPC])
        m["xh"] = np.ascontiguousarray(xh[c * BPC:(c + 1) * BPC])
        in_maps.append(m)
    return in_maps


def kernel(x, Wq, bq, Wk, bk, Wv, bv, rel_h, rel_w, reg_qk, reg_v):
    # reg_qk / reg_v are computed-then-dropped by the reference -> unused.
    in_maps = build_in_maps(x, Wq, bq, Wk, bk, Wv, bv, rel_h, rel_w)

    if "nc" not in _CACHE:
        _CACHE["nc"] = build_bass()
    res = run_bass_kernel_spmd(_CACHE["nc"], in_maps, list(range(N_CORES)))
    outs = [np.asarray(r["out"]) for r in res.results]
    return np.concatenate(outs, axis=0).reshape(B, C, WD, HD)


if __name__ == "__main__":
    nc = build_bass()
    print("built ok")


# revision 22
# speedup vs baseline: 1.3146x; 1.0017x over previous
"""Trainium2 Bass kernel for nn_MHSA_40346922778634.

Math (per batch b, head h; the reference computes-then-drops the register
group, so reg_qk/reg_v are dead inputs):
  X = x[b] as [C=512, N=1024]
  Q = Wq X + bq ; K = Wk X + bk ; V = Wv X + bv       (per head: [64, N])
  P_h = (rel_h + rel_w) reshaped [head, 64, N]
  E[i,j] = Q_h[:,i].K_h[:,j] + P_h[:,i].Q_h[:,j]      ([N, N])
  attn = softmax(E, axis=-1)
  Out_h = V_h @ attn^T ; out[b, h*64:(h+1)*64] = Out_h + X[h*64:(h+1)*64]

Kernel strategy (8 cores, data-parallel over batch, 2 batches/core):
  - fp16 operands for projection + energy matmuls (fp32 runs at 1/4 PE rate;
    fp16 streams at bf16 rate with 8x the mantissa of bf16).
  - Stacked QK projection: one GEMM emits qkall[:, h] = [Q_h; K_h] (stacked
    weight matrix, host-prepped), so the energy's Z operand is a direct
    slice - no per-head assembly copies.
  - E^T = Z^T U with Z = [Q_h; K_h], U = [P_h; Q_h] (row order within the
    stack is irrelevant as long as rows pair K<->Q and Q<->P). U lives in a
    static tile: pos half preloaded once, Q half refreshed per (b, h) by one
    SBUF->SBUF DMA (the partition shift DMA is off the compute engines).
  - exp without max-subtraction (logits bounded, safe in fp32 PSUM),
    T^T = exp(E^T) stored bf16 (range needs bf16, fp16 would overflow).
  - denominator via ones-augmented V^T (padded layout, 65 cols per head with
    the 65th = 1.0), AV matmuls bf16.
  - normalize: den broadcast via rank-1 matmul, reciprocal_approx_fast
    (single-pass custom DVE op - the stock 8-cycle/elem reciprocal was the
    pipeline's serial bottleneck), DVE multiply, GpSimd residual add.
"""

import sys

import numpy as np

try:
    import concourse.bass as bass  # noqa: F401
except Exception:  # pragma: no cover
    sys.path.insert(0, "/opt/trn_rl_repo")

import concourse.bass as bass  # noqa: F401
import concourse.tile as tile
from concourse import bacc, library_config, mybir
from concourse.bass_utils import run_bass_kernel_spmd

F32 = mybir.dt.float32
F16 = mybir.dt.float16
BF16 = mybir.dt.bfloat16
EXP = mybir.ActivationFunctionType.Exp

N_CORES = 8
B, C, WD, HD = 16, 512, 32, 32
HEAD, D, N = 8, 64, 1024
BPC = B // N_CORES  # batches per core


def build_bass():
    nc = bacc.Bacc("TRN2")

    xs_d = nc.dram_tensor("xs", [BPC, C, N], F32, kind="ExternalInput")
    xh_d = nc.dram_tensor("xh", [BPC, C, N], F16, kind="ExternalInput")
    wqkt_d = nc.dram_tensor("wqkt", [4, 128, 1024], F16, kind="ExternalInput")
    wvt_d = nc.dram_tensor("wvt", [4, 128, 512], F16, kind="ExternalInput")
    bqk_d = nc.dram_tensor("bqk", [8, 128, 1], F32, kind="ExternalInput")
    pos_d = nc.dram_tensor("pos", [HEAD, D, N], F16, kind="ExternalInput")
    out_d = nc.dram_tensor("out", [BPC, C, N], F32, kind="ExternalOutput")

    with tile.TileContext(nc) as tc:
        with (
            tc.tile_pool(name="consts", bufs=1) as cpool,
            tc.tile_pool(name="work", bufs=2) as wpool,
            tc.tile_pool(name="psume", bufs=2, space="PSUM") as pse,
            tc.tile_pool(name="psumo", bufs=2, space="PSUM") as pso,
        ):
            # ---- batch-0 X + weights, emission-interleaved so the first
            # projection's operands land first in the DMA queues ----
            wqkt_sb = cpool.tile([128, 4, 1024], F16, name="wqkt_sb")
            wvt_sb = cpool.tile([128, 4, 512], F16, name="wvt_sb")
            bqk_sb = cpool.tile([128, 8, 1], F32, name="bqk_sb")

            def prep_x(b, wq_too=False):
                x_sb = wpool.tile([128, 4, N], F16, name=f"x_{b}", tag="x")
                for kc in range(4):
                    if wq_too:
                        nc.sync.dma_start(wqkt_sb[:, kc, :], wqkt_d[kc])
                    nc.sync.dma_start(x_sb[:, kc, :], xh_d[b, kc * 128:(kc + 1) * 128, :])
                return x_sb

            ctx = {0: {}}
            ctx[0]["x"] = prep_x(0, wq_too=True)
            for h in range(8):
                nc.sync.dma_start(bqk_sb[:, h, :], bqk_d[h])

            # static per-head U tiles: pos in partitions 0-63 (loaded once),
            # Q_h refreshed into partitions 64-127 each (b, h).
            u_all = cpool.tile([128, 8, N], F16, name="u_all")
            nc.sync.dma_start(u_all[0:64, 0, :], pos_d[0])
            for kc in range(4):
                nc.sync.dma_start(wvt_sb[:, kc, :], wvt_d[kc])
            for h in range(1, 8):
                nc.sync.dma_start(u_all[0:64, h, :], pos_d[h])
            zbias = cpool.tile([128, 1], F32, name="zbias")
            nc.vector.memset(zbias[:], 0.0)
            onesbf = cpool.tile([65, 64], BF16, name="onesbf")
            nc.vector.memset(onesbf[:], 1.0)

            def emit_qkproj_slot(b, h):
                # stacked QK projection slot h: qkall[:, h] = [Q_h; K_h]
                # (f16, bias added): 2 psum groups of 4 accumulating matmuls.
                cx = ctx[b]
                if "qkall" not in cx:
                    cx["qkall"] = wpool.tile(
                        [128, 8, N], F16, name=f"qkall_{b}", tag="qkall", bufs=2
                    )
                qkall = cx["qkall"]
                for nh in range(2):
                    ps = pso.tile([128, 512], F32, name=f"ps_p{b}{h}{nh}", tag="pso")
                    for kc in range(4):
                        nc.tensor.matmul(
                            ps[:],
                            wqkt_sb[:, kc, h * 128:(h + 1) * 128],
                            cx["x"][:, kc, nh * 512:(nh + 1) * 512],
                            start=(kc == 0),
                            stop=(kc == 3),
                        )
                    nc.vector.tensor_scalar_add(
                        qkall[:, h, nh * 512:(nh + 1) * 512], ps[:], bqk_sb[:, h, :]
                    )

            def emit_u(b, h):
                # U_h bottom half <- Q_h (partitions 0-63 of qkall slot h ->
                # partitions 64-127 of u_all slot h): partition shift, so DMA.
                nc.sync.dma_start(u_all[64:128, h, :], ctx[b]["qkall"][0:64, h, :])

            def emit_vproj(b, c0, c1):
                # V^T projection, bf16 out, weights only. The V bias is
                # folded into the residual host-side (num = rawAV + bv*den,
                # so out = rawAV/den + bv); the 65th ones-column per head
                # (softmax denominator) is memset once per vpt tile.
                cx = ctx[b]
                if "vpt" not in cx:
                    cx["vpt"] = wpool.tile([128, 8, 8, 65], BF16, name=f"vpt_{b}", tag="vpt")
                    nc.vector.memset(cx["vpt"][:, :, :, 64], 1.0)
                vpt = cx["vpt"]
                for nc8 in range(c0, c1):
                    ps = pso.tile([128, 8, 64], F32, name=f"ps_v{b}{nc8}", tag="pso")
                    for kc in range(4):
                        nc.tensor.matmul(
                            ps[:, :, :],
                            cx["x"][:, kc, nc8 * 128:(nc8 + 1) * 128],
                            wvt_sb[:, kc, :],
                            start=(kc == 0),
                            stop=(kc == 3),
                        )
                    nc.vector.tensor_copy(vpt[:, nc8, :, 0:64], ps[:, :, :])

            tt_of = {}

            def emit_E_chunk(b, h, j):
                # E^T chunk j + exp. Z = qkall slot h (direct slice),
                # U = u_all slot h (pos + DMA'd Q half).
                qkall = ctx[b]["qkall"]
                eps = pse.tile([128, N], F32, name=f"ps_e{b}{h}{j}", tag="pse")
                for ih in range(2):
                    nc.tensor.matmul(
                        eps[:, ih * 512:(ih + 1) * 512],
                        qkall[:, h, j * 128:(j + 1) * 128],
                        u_all[:, h, ih * 512:(ih + 1) * 512],
                        start=True,
                        stop=True,
                    )
                tt = wpool.tile([128, N], BF16, name=f"tt_{b}_{h}_{j}", tag="tt", bufs=8)
                nc.scalar.activation(tt[:], eps[:], EXP, bias=zbias[:])
                tt_of[(b, h, j)] = tt

            ops_of = {}

            def emit_AV_chunk(b, h, j):
                # AV accumulation for m-chunk j. Emitted LAG global chunks
                # behind the E/exp emission: the eps double-buffer already
                # forces exp(ci-2) done before E(ci) issues, so by the time
                # the PE's strict-FIFO queue reaches this matmul its tt is
                # ready and the engine never stalls on ACT.
                cx = ctx[b]
                if j == 0:
                    ops_of[(b, h)] = pso.tile([65, N], F32, name=f"ps_o{b}{h}", tag="pso")
                ops = ops_of[(b, h)]
                tt = tt_of.pop((b, h, j))
                for mh in range(2):
                    nc.tensor.matmul(
                        ops[:, mh * 512:(mh + 1) * 512],
                        cx["vpt"][:, j, h, :],
                        tt[:, mh * 512:(mh + 1) * 512],
                        start=(j == 0),
                        stop=(j == 7),
                    )

            def emit_norm(b, h, last=False):
                # normalize: den -> bf16 row, rank-1 broadcast matmul,
                # single-pass approx reciprocal, multiply, residual-add
                # (GpSimd normally; DVE for the final head to cut the tail),
                # store fp32.
                ops = ops_of.pop((b, h))
                dhi = wpool.tile([65, N], BF16, name=f"dhi_{b}_{h}", tag="dhi")
                nc.vector.tensor_copy(dhi[64:65, :], ops[64:65, :])
                rps = pse.tile([64, N], F32, name=f"ps_r{b}{h}", tag="pse")
                for mh in range(2):
                    nc.tensor.matmul(
                        rps[:, mh * 512:(mh + 1) * 512],
                        onesbf[64:65, 0:64],
                        dhi[64:65, mh * 512:(mh + 1) * 512],
                        start=True,
                        stop=True,
                    )
                rbinv = wpool.tile([64, N], F32, name=f"rbinv_{b}_{h}", tag="rbinv")
                nc.vector.reciprocal_approx_fast(out=rbinv[:], in_=rps[:])
                osb = wpool.tile([64, N], F32, name=f"osb_{b}_{h}", tag="ostage", bufs=3)
                nc.vector.tensor_mul(osb[:], ops[0:64, :], rbinv[:])
                xres = wpool.tile([64, N], F32, name=f"xres_{b}_{h}", tag="xres")
                nc.sync.dma_start(xres[:], xs_d[b, h * 64:(h + 1) * 64, :])
                fin = wpool.tile([64, N], F32, name=f"fin_{b}_{h}", tag="ostage", bufs=3)
                if last:
                    nc.vector.tensor_add(fin[:], osb[:], xres[:])
                else:
                    nc.gpsimd.tensor_add(fin[:], osb[:], xres[:])
                nc.sync.dma_start(out_d[b, h * 64:(h + 1) * 64, :], fin[:])

            # ---- software pipeline over (batch, head): per head the
            # energy/exp/AV chunks are fused; the next head's projection
            # slot + U refresh are emitted at the end of each head so their
            # PE work fills the ACT-bound gaps; next batch's X load +
            # projections are emitted mid-way through the previous batch ----
            done = set()

            def proj_slot_once(b, h):
                if ("p", b, h) not in done:
                    done.add(("p", b, h))
                    emit_qkproj_slot(b, h)

            def u_once(b, h):
                if ("u", b, h) not in done:
                    done.add(("u", b, h))
                    emit_u(b, h)

            proj_slot_once(0, 0)
            u_once(0, 0)
            LAG = 4  # AV chunks trail E/exp chunks by this many global chunks
            av_pend = []

            def pop_av():
                pb, ph, pj = av_pend.pop(0)
                emit_AV_chunk(pb, ph, pj)
                if pj == 7:
                    emit_norm(pb, ph, last=(pb == BPC - 1 and ph == 7))

            emit_vproj(0, 0, 4)
            for b in range(BPC):
                for h in range(8):
                    for j in range(8):
                        if b == 0 and h == 0 and j >= 4:
                            emit_vproj(0, j, j + 1)
                        emit_E_chunk(b, h, j)
                        av_pend.append((b, h, j))
                        if len(av_pend) > LAG:
                            pop_av()
                        if j == 3 and h < 7:
                            # mid-head prefetch: next head's projection slot
                            # + U refresh, so the proj->bias->DMA chain
                            # completes before the next head's first energy
                            proj_slot_once(b, h + 1)
                            u_once(b, h + 1)
                    if b + 1 < BPC:
                        if h == 4:
                            ctx[b + 1] = {"x": prep_x(b + 1)}
                            for hh in range(8):
                                proj_slot_once(b + 1, hh)
                            # only heads whose energy(b, hh) is already
                            # emitted may have their U slot overwritten --
                            # Tile orders by emission, so emitting early
                            # would invert the WAR dependency.
                            for hh in range(5):
                                u_once(b + 1, hh)
                        elif h in (5, 6, 7):
                            u_once(b + 1, h)
                            emit_vproj(b + 1, 3 * (h - 5), min(8, 3 * (h - 5) + 3))
            while av_pend:
                pop_av()

    nc.compile()
    return nc


def _prep_consts(Wq, bq, Wk, bk, Wv, bv, rel_h, rel_w):
    # stacked QK weight: output slot h covers channels [Wq head h; Wk head h]
    wstack = np.empty((1024, 512), np.float32)
    bstack = np.empty((8, 128, 1), np.float32)
    for h in range(HEAD):
        wstack[h * 128:h * 128 + 64] = Wq[h * 64:(h + 1) * 64]
        wstack[h * 128 + 64:h * 128 + 128] = Wk[h * 64:(h + 1) * 64]
        bstack[h, 0:64, 0] = bq[h * 64:(h + 1) * 64]
        bstack[h, 64:128, 0] = bk[h * 64:(h + 1) * 64]
    wqkt = np.ascontiguousarray(wstack.T).reshape(4, 128, 1024).astype(np.float16)
    wvt = np.ascontiguousarray(Wv.T).reshape(4, 128, 512).astype(np.float16)
    pos = (rel_h + rel_w).reshape(HEAD, D, N).astype(np.float16)
    return {
        "wqkt": wqkt,
        "wvt": wvt,
        "bqk": bstack,
        "pos": pos,
    }


_CACHE = {}


def build_in_maps(x, Wq, bq, Wk, bk, Wv, bv, rel_h, rel_w):
    x = np.asarray(x, np.float32)
    consts = _prep_consts(
        *[np.asarray(a, np.float32) for a in (Wq, bq, Wk, bk, Wv, bv, rel_h, rel_w)]
    )
    xr = x.reshape(B, C, N)
    xh = xr.astype(np.float16)
    # V bias folded into the residual: out = rawAV/den + bv + x
    xres = xr + np.asarray(bv, np.float32)[None, :, None]
    in_maps = []
    for c in range(N_CORES):
        m = dict(consts)
        m["xs"] = np.ascontiguousarray(xres[c * BPC:(c + 1) * BPC])
        m["xh"] = np.ascontiguousarray(xh[c * BPC:(c + 1) * BPC])
        in_maps.append(m)
    return in_maps


def kernel(x, Wq, bq, Wk, bk, Wv, bv, rel_h, rel_w, reg_qk, reg_v):
    # reg_qk / reg_v are computed-then-dropped by the reference -> unused.
    in_maps = build_in_maps(x, Wq, bq, Wk, bk, Wv, bv, rel_h, rel_w)

    if "nc" not in _CACHE:
        _CACHE["nc"] = build_bass()
    res = run_bass_kernel_spmd(_CACHE["nc"], in_maps, list(range(N_CORES)))
    outs = [np.asarray(r["out"]) for r in res.results]
    return np.concatenate(outs, axis=0).reshape(B, C, WD, HD)


if __name__ == "__main__":
    nc = build_bass()
    print("built ok")


# revision 25
# speedup vs baseline: 1.3360x; 1.0163x over previous
"""Trainium2 Bass kernel for nn_MHSA_40346922778634.

Math (per batch b, head h; the reference computes-then-drops the register
group, so reg_qk/reg_v are dead inputs):
  X = x[b] as [C=512, N=1024]
  Q = Wq X + bq ; K = Wk X + bk ; V = Wv X + bv       (per head: [64, N])
  P_h = (rel_h + rel_w) reshaped [head, 64, N]
  E[i,j] = Q_h[:,i].K_h[:,j] + P_h[:,i].Q_h[:,j]      ([N, N])
  attn = softmax(E, axis=-1)
  Out_h = V_h @ attn^T ; out[b, h*64:(h+1)*64] = Out_h + X[h*64:(h+1)*64]

Kernel strategy (8 cores, data-parallel over batch, 2 batches/core):
  - fp16 operands for projection + energy matmuls (fp32 runs at 1/4 PE rate;
    fp16 streams at bf16 rate with 8x the mantissa of bf16).
  - Stacked QK projection: one GEMM emits qkall[:, h] = [Q_h; K_h] (stacked
    weight matrix, host-prepped), so the energy's Z operand is a direct
    slice - no per-head assembly copies.
  - E^T = Z^T U with Z = [Q_h; K_h], U = [P_h; Q_h] (row order within the
    stack is irrelevant as long as rows pair K<->Q and Q<->P). U lives in a
    static tile: pos half preloaded once, Q half refreshed per (b, h) by one
    SBUF->SBUF DMA (the partition shift DMA is off the compute engines).
  - exp without max-subtraction (logits bounded, safe in fp32 PSUM),
    T^T = exp(E^T) stored bf16 (range needs bf16, fp16 would overflow).
  - denominator via ones-augmented V^T (padded layout, 65 cols per head with
    the 65th = 1.0), AV matmuls bf16.
  - normalize: den broadcast via rank-1 matmul, reciprocal_approx_fast
    (single-pass custom DVE op - the stock 8-cycle/elem reciprocal was the
    pipeline's serial bottleneck), DVE multiply, GpSimd residual add.
"""

import sys

import numpy as np

try:
    import concourse.bass as bass  # noqa: F401
except Exception:  # pragma: no cover
    sys.path.insert(0, "/opt/trn_rl_repo")

import concourse.bass as bass  # noqa: F401
import concourse.tile as tile
from concourse import bacc, library_config, mybir
from concourse.bass_utils import run_bass_kernel_spmd

F32 = mybir.dt.float32
F16 = mybir.dt.float16
BF16 = mybir.dt.bfloat16
EXP = mybir.ActivationFunctionType.Exp

N_CORES = 8
B, C, WD, HD = 16, 512, 32, 32
HEAD, D, N = 8, 64, 1024
BPC = B // N_CORES  # batches per core


def build_bass():
    nc = bacc.Bacc("TRN2")

    xs_d = nc.dram_tensor("xs", [BPC, C, N], F32, kind="ExternalInput")
    xh_d = nc.dram_tensor("xh", [BPC, C, N], F16, kind="ExternalInput")
    wqkt_d = nc.dram_tensor("wqkt", [4, 128, 1024], F16, kind="ExternalInput")
    wvt_d = nc.dram_tensor("wvt", [4, 128, 512], F16, kind="ExternalInput")
    bqk_d = nc.dram_tensor("bqk", [8, 128, 1], F32, kind="ExternalInput")
    pos_d = nc.dram_tensor("pos", [HEAD, D, N], F16, kind="ExternalInput")
    out_d = nc.dram_tensor("out", [BPC, C, N], F32, kind="ExternalOutput")

    with tile.TileContext(nc) as tc:
        with (
            tc.tile_pool(name="consts", bufs=1) as cpool,
            tc.tile_pool(name="work", bufs=2) as wpool,
            tc.tile_pool(name="psume", bufs=2, space="PSUM") as pse,
            tc.tile_pool(name="psumo", bufs=2, space="PSUM") as pso,
        ):
            # ---- batch-0 X + weights, emission-interleaved so the first
            # projection's operands land first in the DMA queues ----
            wqkt_sb = cpool.tile([128, 4, 1024], F16, name="wqkt_sb")
            wvt_sb = cpool.tile([128, 4, 512], F16, name="wvt_sb")
            bqk_sb = cpool.tile([128, 8, 1], F32, name="bqk_sb")

            def prep_x(b, wq_too=False):
                x_sb = wpool.tile([128, 4, N], F16, name=f"x_{b}", tag="x")
                for kc in range(4):
                    if wq_too:
                        nc.sync.dma_start(wqkt_sb[:, kc, :], wqkt_d[kc])
                    nc.sync.dma_start(x_sb[:, kc, :], xh_d[b, kc * 128:(kc + 1) * 128, :])
                return x_sb

            ctx = {0: {}}
            ctx[0]["x"] = prep_x(0, wq_too=True)
            for h in range(8):
                nc.sync.dma_start(bqk_sb[:, h, :], bqk_d[h])

            # static per-head U tiles: pos in partitions 0-63 (loaded once),
            # Q_h refreshed into partitions 64-127 each (b, h).
            u_all = cpool.tile([128, 8, N], F16, name="u_all")
            nc.sync.dma_start(u_all[0:64, 0, :], pos_d[0])
            for kc in range(4):
                nc.sync.dma_start(wvt_sb[:, kc, :], wvt_d[kc])
            for h in range(1, 8):
                nc.sync.dma_start(u_all[0:64, h, :], pos_d[h])
            zbias = cpool.tile([128, 1], F32, name="zbias")
            nc.vector.memset(zbias[:], 0.0)
            onesbf = cpool.tile([65, 64], BF16, name="onesbf")
            nc.vector.memset(onesbf[:], 1.0)

            # PE warm-up: dependency-free matmuls on a zeroed tile run during
            # the initial DMA ramp, so HAM un-throttles before real work and
            # the first projection matmuls issue at full clock.
            warm = cpool.tile([128, 512], F16, name="warm")
            nc.vector.memset(warm[:], 0.0)
            wps = pse.tile([128, 512], F32, name="ps_warm", tag="pse")
            for _ in range(10):
                nc.tensor.matmul(wps[:], warm[:, 0:128], warm[:], start=True, stop=True)

            def emit_qkproj_slot(b, h):
                # stacked QK projection slot h: qkall[:, h] = [Q_h; K_h]
                # (f16, bias added): 2 psum groups of 4 accumulating matmuls.
                cx = ctx[b]
                if "qkall" not in cx:
                    cx["qkall"] = wpool.tile(
                        [128, 8, N], F16, name=f"qkall_{b}", tag="qkall", bufs=2
                    )
                qkall = cx["qkall"]
                for nh in range(2):
                    ps = pso.tile([128, 512], F32, name=f"ps_p{b}{h}{nh}", tag="pso")
                    for kc in range(4):
                        nc.tensor.matmul(
                            ps[:],
                            wqkt_sb[:, kc, h * 128:(h + 1) * 128],
                            cx["x"][:, kc, nh * 512:(nh + 1) * 512],
                            start=(kc == 0),
                            stop=(kc == 3),
                        )
                    nc.vector.tensor_scalar_add(
                        qkall[:, h, nh * 512:(nh + 1) * 512], ps[:], bqk_sb[:, h, :]
                    )

            def emit_u(b, h):
                # U_h bottom half <- Q_h (partitions 0-63 of qkall slot h ->
                # partitions 64-127 of u_all slot h): partition shift, so DMA.
                nc.sync.dma_start(u_all[64:128, h, :], ctx[b]["qkall"][0:64, h, :])

            def emit_vproj(b, c0, c1):
                # V^T projection, bf16 out, weights only. The V bias is
                # folded into the residual host-side (num = rawAV + bv*den,
                # so out = rawAV/den + bv); the 65th ones-column per head
                # (softmax denominator) is memset once per vpt tile.
                cx = ctx[b]
                if "vpt" not in cx:
                    cx["vpt"] = wpool.tile([128, 8, 8, 65], BF16, name=f"vpt_{b}", tag="vpt")
                    nc.vector.memset(cx["vpt"][:, :, :, 64], 1.0)
                vpt = cx["vpt"]
                for nc8 in range(c0, c1):
                    ps = pso.tile([128, 8, 64], F32, name=f"ps_v{b}{nc8}", tag="pso")
                    for kc in range(4):
                        nc.tensor.matmul(
                            ps[:, :, :],
                            cx["x"][:, kc, nc8 * 128:(nc8 + 1) * 128],
                            wvt_sb[:, kc, :],
                            start=(kc == 0),
                            stop=(kc == 3),
                        )
                    nc.vector.tensor_copy(vpt[:, nc8, :, 0:64], ps[:, :, :])

            tt_of = {}

            def emit_E_chunk(b, h, j):
                # E^T chunk j + exp. Z = qkall slot h (direct slice),
                # U = u_all slot h (pos + DMA'd Q half).
                qkall = ctx[b]["qkall"]
                eps = pse.tile([128, N], F32, name=f"ps_e{b}{h}{j}", tag="pse")
                for ih in range(2):
                    nc.tensor.matmul(
                        eps[:, ih * 512:(ih + 1) * 512],
                        qkall[:, h, j * 128:(j + 1) * 128],
                        u_all[:, h, ih * 512:(ih + 1) * 512],
                        start=True,
                        stop=True,
                    )
                tt = wpool.tile([128, N], BF16, name=f"tt_{b}_{h}_{j}", tag="tt", bufs=8)
                nc.scalar.activation(tt[:], eps[:], EXP, bias=zbias[:])
                tt_of[(b, h, j)] = tt

            ops_of = {}

            def emit_AV_chunk(b, h, j):
                # AV accumulation for m-chunk j. Emitted LAG global chunks
                # behind the E/exp emission: the eps double-buffer already
                # forces exp(ci-2) done before E(ci) issues, so by the time
                # the PE's strict-FIFO queue reaches this matmul its tt is
                # ready and the engine never stalls on ACT.
                cx = ctx[b]
                if j == 0:
                    ops_of[(b, h)] = pso.tile([65, N], F32, name=f"ps_o{b}{h}", tag="pso")
                ops = ops_of[(b, h)]
                tt = tt_of.pop((b, h, j))
                for mh in range(2):
                    nc.tensor.matmul(
                        ops[:, mh * 512:(mh + 1) * 512],
                        cx["vpt"][:, j, h, :],
                        tt[:, mh * 512:(mh + 1) * 512],
                        start=(j == 0),
                        stop=(j == 7),
                    )

            def emit_norm(b, h, last=False):
                # normalize: den -> bf16 row, rank-1 broadcast matmul,
                # single-pass approx reciprocal, multiply, residual-add
                # (GpSimd normally; DVE for the final head to cut the tail),
                # store fp32.
                ops = ops_of.pop((b, h))
                dhi = wpool.tile([65, N], BF16, name=f"dhi_{b}_{h}", tag="dhi")
                nc.vector.tensor_copy(dhi[64:65, :], ops[64:65, :])
                rps = pse.tile([64, N], F32, name=f"ps_r{b}{h}", tag="pse")
                for mh in range(2):
                    nc.tensor.matmul(
                        rps[:, mh * 512:(mh + 1) * 512],
                        onesbf[64:65, 0:64],
                        dhi[64:65, mh * 512:(mh + 1) * 512],
                        start=True,
                        stop=True,
                    )
                rbinv = wpool.tile([64, N], F32, name=f"rbinv_{b}_{h}", tag="rbinv")
                nc.vector.reciprocal_approx_fast(out=rbinv[:], in_=rps[:])
                osb = wpool.tile([64, N], F32, name=f"osb_{b}_{h}", tag="ostage", bufs=3)
                nc.vector.tensor_mul(osb[:], ops[0:64, :], rbinv[:])
                xres = wpool.tile([64, N], F32, name=f"xres_{b}_{h}", tag="xres")
                nc.sync.dma_start(xres[:], xs_d[b, h * 64:(h + 1) * 64, :])
                fin = wpool.tile([64, N], F32, name=f"fin_{b}_{h}", tag="ostage", bufs=3)
                if last:
                    nc.vector.tensor_add(fin[:], osb[:], xres[:])
                else:
                    nc.gpsimd.tensor_add(fin[:], osb[:], xres[:])
                nc.sync.dma_start(out_d[b, h * 64:(h + 1) * 64, :], fin[:])

            # ---- software pipeline over (batch, head): per head the
            # energy/exp/AV chunks are fused; the next head's projection
            # slot + U refresh are emitted at the end of each head so their
            # PE work fills the ACT-bound gaps; next batch's X load +
            # projections are emitted mid-way through the previous batch ----
            done = set()

            def proj_slot_once(b, h):
                if ("p", b, h) not in done:
                    done.add(("p", b, h))
                    emit_qkproj_slot(b, h)

            def u_once(b, h):
                if ("u", b, h) not in done:
                    done.add(("u", b, h))
                    emit_u(b, h)

            proj_slot_once(0, 0)
            u_once(0, 0)
            LAG = 4  # AV chunks trail E/exp chunks by this many global chunks
            av_pend = []

            def pop_av():
                pb, ph, pj = av_pend.pop(0)
                emit_AV_chunk(pb, ph, pj)
                if pj == 7:
                    emit_norm(pb, ph, last=(pb == BPC - 1 and ph == 7))

            emit_vproj(0, 0, 4)
            for b in range(BPC):
                for h in range(8):
                    for j in range(8):
                        if b == 0 and h == 0 and j >= 4:
                            emit_vproj(0, j, j + 1)
                        emit_E_chunk(b, h, j)
                        av_pend.append((b, h, j))
                        if len(av_pend) > LAG:
                            pop_av()
                        if j == 3 and h < 7:
                            # mid-head prefetch: next head's projection slot
                            # + U refresh, so the proj->bias->DMA chain
                            # completes before the next head's first energy
                            proj_slot_once(b, h + 1)
                            u_once(b, h + 1)
                    if b + 1 < BPC:
                        if h == 4:
                            ctx[b + 1] = {"x": prep_x(b + 1)}
                            for hh in range(8):
                                proj_slot_once(b + 1, hh)
                            # only heads whose energy(b, hh) is already
                            # emitted may have their U slot overwritten --
                            # Tile orders by emission, so emitting early
                            # would invert the WAR dependency.
                            for hh in range(5):
                                u_once(b + 1, hh)
                        elif h in (5, 6, 7):
                            u_once(b + 1, h)
                            emit_vproj(b + 1, 3 * (h - 5), min(8, 3 * (h - 5) + 3))
            while av_pend:
                pop_av()

    nc.compile()
    return nc


def _prep_consts(Wq, bq, Wk, bk, Wv, bv, rel_h, rel_w):
    # stacked QK weight: output slot h covers channels [Wq head h; Wk head h]
    wstack = np.empty((1024, 512), np.float32)
    bstack = np.empty((8, 128, 1), np.float32)
    for h in range(HEAD):
        wstack[h * 128:h * 128 + 64] = Wq[h * 64:(h + 1) * 64]
        wstack[h * 128 + 64:h * 128 + 128] = Wk[h * 64:(h + 1) * 64]
        bstack[h, 0:64, 0] = bq[h * 64:(h + 1) * 64]
        bstack[h, 64:128, 0] = bk[h * 64:(h + 1) * 64]
    wqkt = np.ascontiguousarray(wstack.T).reshape(4, 128, 1024).astype(np.float16)
    wvt = np.ascontiguousarray(Wv.T).reshape(4, 128, 512).astype(np.float16)
    pos = (rel_h + rel_w).reshape(HEAD, D, N).astype(np.float16)
    return {
        "wqkt": wqkt,
        "wvt": wvt,
        "bqk": bstack,
        "pos": pos,
    }


_CACHE = {}


def build_in_maps(x, Wq, bq, Wk, bk, Wv, bv, rel_h, rel_w):
    x = np.asarray(x, np.float32)
    consts = _prep_consts(
        *[np.asarray(a, np.float32) for a in (Wq, bq, Wk, bk, Wv, bv, rel_h, rel_w)]
    )
    xr = x.reshape(B, C, N)
    xh = xr.astype(np.float16)
    # V bias folded into the residual: out = rawAV/den + bv + x
    xres = xr + np.asarray(bv, np.float32)[None, :, None]
    in_maps = []
    for c in range(N_CORES):
        m = dict(consts)
        m["xs"] = np.ascontiguousarray(xres[c * BPC:(c + 1) * BPC])
        m["xh"] = np.ascontiguousarray(xh[c * BPC:(c + 1) * BPC])
        in_maps.append(m)
    return in_maps


def kernel(x, Wq, bq, Wk, bk, Wv, bv, rel_h, rel_w, reg_qk, reg_v):
    # reg_qk / reg_v are computed-then-dropped by the reference -> unused.
    in_maps = build_in_maps(x, Wq, bq, Wk, bk, Wv, bv, rel_h, rel_w)

    if "nc" not in _CACHE:
        _CACHE["nc"] = build_bass()
    res = run_bass_kernel_spmd(_CACHE["nc"], in_maps, list(range(N_CORES)))
    outs = [np.asarray(r["out"]) for r in res.results]
    return np.concatenate(outs, axis=0).reshape(B, C, WD, HD)


if __name__ == "__main__":
    nc = build_bass()
    print("built ok")
